# revision 6
# baseline (speedup 1.0000x reference)
"""Bass/Trainium2 kernel for nn_DifferentialAttentionLayer (moe_routing).

Strategy: data-parallel over batch across the 8 NeuronCores (4 samples each).
The tiny router (mean-pool -> 2-layer MLP -> gumbel softmax -> top-16 of a
32-head pool) runs on host CPU with jax to reproduce the reference top_k
bit-exactly; the selected per-sample head weights are gathered/packed on host
and the heavy compute (QKV projections, attention, softmax, entropy, output
projection, residual layernorm) runs on-device in a single SPMD Bass program.

Device-side layout notes (per sample):
  - qT/kT [1024, 512]  (head-major rows: m = 64*h + k), computed as
    Wq[d,m]^T @ xT[d,s] with fp32r matmuls.
  - v [512, 1040]: 16 groups of 65 cols (64 v-cols + a ones col) so the
    "ho" matmul v_aug^T @ expT also produces the softmax row-sum S.
  - attention is computed transposed: scoresT[t,s] = k_h^T q_h, softmax
    without max-subtraction (scores are O(+-3), fp32-exact), entropy via
    ent = ln S - (sum_t e^l * l_raw)/(8 S) shipped to host as S and D rows.
  - mhT [1024, 512] is assembled directly in transposed form so the output
    projection needs no transposes anywhere.
"""

import numpy as np

B, S, D, H, POOL, DK = 32, 512, 1024, 16, 32, 64
NCORES = 8
NSAMP = B // NCORES
TEMP = 0.5

_PROGRAM_CACHE = {}


# ----------------------------------------------------------------- host router
def _router_topk(x, u, r1_w, r1_b, ln_g, ln_b, r2_w, r2_b):
    """Reproduce the reference routing exactly (jax on CPU)."""
    import jax
    import jax.numpy as jnp

    cpu = jax.devices("cpu")[0]
    with jax.default_device(cpu):
        xj = jnp.asarray(x)
        pooled = jnp.mean(xj, axis=1)
        h = pooled @ jnp.asarray(r1_w) + jnp.asarray(r1_b)
        mu = jnp.mean(h, axis=-1, keepdims=True)
        var = jnp.mean((h - mu) ** 2, axis=-1, keepdims=True)
        h = (h - mu) / jnp.sqrt(var + 1e-5) * jnp.asarray(ln_g) + jnp.asarray(ln_b)
        h = jax.nn.relu(h)
        rs = h @ jnp.asarray(r2_w) + jnp.asarray(r2_b)
        gumbel = -jnp.log(-jnp.log(jnp.asarray(u)))
        rw = jax.nn.softmax((rs + gumbel) / TEMP, axis=-1)
        _, idx = jax.lax.top_k(rw, H)
        return np.asarray(idx)


# ------------------------------------------------------------- device program
def _build_program(nsamp):
    import concourse.bacc as bacc
    import concourse.mybir as mybir
    from concourse import tile

    f32 = mybir.dt.float32
    f32r = mybir.dt.float32r
    AF = mybir.ActivationFunctionType
    ALU = mybir.AluOpType

    nc = bacc.Bacc("TRN2", target_bir_lowering=False)

    xT_d = nc.declare_dram_parameter("xT", [nsamp, D, S], f32, isOutput=False)
    xn_d = nc.declare_dram_parameter("xn", [nsamp, S, D], f32, isOutput=False)
    wq_d = nc.declare_dram_parameter("wq", [nsamp, 8, 128, D], f32, isOutput=False)
    wk_d = nc.declare_dram_parameter("wk", [nsamp, 8, 128, D], f32, isOutput=False)
    wv_d = nc.declare_dram_parameter("wv", [nsamp, 2, 128, 4096], f32, isOutput=False)
    bq_d = nc.declare_dram_parameter("bq", [nsamp, 128, 8], f32, isOutput=False)
    bk_d = nc.declare_dram_parameter("bk", [nsamp, 128, 8], f32, isOutput=False)
    bv_d = nc.declare_dram_parameter("bv", [nsamp, 1, D], f32, isOutput=False)
    ow_d = nc.declare_dram_parameter("ow", [2, 128, 4096], f32, isOutput=False)
    # const4 rows: 0 = out_b, 1 = norm_g, 2 = norm_b
    c4_d = nc.declare_dram_parameter("c4", [4, D], f32, isOutput=False)

    out_d = nc.declare_dram_parameter("out", [nsamp, S, D], f32, isOutput=True)
    ent_d = nc.declare_dram_parameter("ent", [nsamp, H, 2, S], f32, isOutput=True)

    from contextlib import ExitStack

    lp = nc.allow_low_precision("fp32r matmul operands throughout")
    lp.__enter__()
    with tile.TileContext(nc) as tc:
        with ExitStack() as stack:
            pw = stack.enter_context(tc.tile_pool(name="pw", bufs=2))
            pwv = stack.enter_context(tc.tile_pool(name="pwv", bufs=1))
            pow_ = stack.enter_context(tc.tile_pool(name="pow", bufs=2))
            pxT = stack.enter_context(tc.tile_pool(name="pxT", bufs=8))
            pxn = stack.enter_context(tc.tile_pool(name="pxn", bufs=1))
            pqT = stack.enter_context(tc.tile_pool(name="pqT", bufs=8))
            pkT = stack.enter_context(tc.tile_pool(name="pkT", bufs=8))
            pv = stack.enter_context(tc.tile_pool(name="pv", bufs=4))
            pmh = stack.enter_context(tc.tile_pool(name="pmh", bufs=8))
            pet = stack.enter_context(tc.tile_pool(name="pet", bufs=3))
            ppr = stack.enter_context(tc.tile_pool(name="ppr", bufs=2))
            phos = stack.enter_context(tc.tile_pool(name="phos", bufs=2))
            pln = stack.enter_context(tc.tile_pool(name="pln", bufs=3))
            psm = stack.enter_context(tc.tile_pool(name="psm", bufs=2))
            pcn = stack.enter_context(tc.tile_pool(name="pcn", bufs=1))
            ps = stack.enter_context(tc.tile_pool(name="ps", bufs=8, space="PSUM"))
            # ---------------- constants
            ones1_f = pcn.tile([1, 128], f32, tag="ones1f")
            nc.vector.memset(ones1_f[:], 1.0)
            ones1 = pcn.tile([1, 128], f32r, tag="ones1")
            nc.vector.tensor_copy(ones1[:], ones1_f[:])
            ones128_f = pcn.tile([128, 1], f32, tag="o128f")
            nc.vector.memset(ones128_f[:], 1.0)
            ones128 = pcn.tile([128, 1], f32r, tag="o128")
            nc.vector.tensor_copy(ones128[:], ones128_f[:])
            onesv = pcn.tile([128, 16], f32, tag="onesv")
            nc.vector.memset(onesv[:], 1.0)
            eps_t = pcn.tile([128, 1], f32, tag="eps")
            nc.vector.memset(eps_t[:], 1e-5)

            cob = pcn.tile([1, D], f32r, tag="cob")
            nc.sync.dma_start(out=cob[:], in_=c4_d[0:1, :].bitcast(f32r))
            cng = pcn.tile([1, D], f32r, tag="cng")
            nc.sync.dma_start(out=cng[:], in_=c4_d[1:2, :].bitcast(f32r))
            cnb = pcn.tile([1, D], f32r, tag="cnb")
            nc.sync.dma_start(out=cnb[:], in_=c4_d[2:3, :].bitcast(f32r))

            # G/B broadcast tiles [128, D] of norm_g / norm_b
            G = pcn.tile([128, D], f32, tag="G")
            Bb = pcn.tile([128, D], f32, tag="B")
            for src, dst in ((cng, G), (cnb, Bb)):
                for c2 in range(2):
                    p = ps.tile([128, 512], f32, tag="ps")
                    nc.tensor.matmul(
                        p[:], ones1[:, 0:128], src[:, c2 * 512 : (c2 + 1) * 512],
                        start=True, stop=True,
                    )
                    nc.scalar.copy(dst[:, c2 * 512 : (c2 + 1) * 512], p[:])

            # out_w resident (packed per n-half)
            ow = []
            for n2 in range(2):
                t = pow_.tile([128, 4096], f32r, tag="ow")
                nc.sync.dma_start(out=t[:], in_=ow_d[n2].bitcast(f32r))
                ow.append(t)

            # ---------------- per-sample pipeline
            for b in range(nsamp):
                bq = psm.tile([128, 8], f32, tag="bq")
                nc.sync.dma_start(out=bq[:], in_=bq_d[b])
                bk = psm.tile([128, 8], f32, tag="bk")
                nc.sync.dma_start(out=bk[:], in_=bk_d[b])
                bv = psm.tile([1, D], f32r, tag="bv")
                nc.sync.dma_start(out=bv[:], in_=bv_d[b].bitcast(f32r))

                xt = []
                for d in range(8):
                    t = pxT.tile([128, S], f32r, tag="xT")
                    nc.sync.dma_start(
                        out=t[:], in_=xT_d[b, d * 128 : (d + 1) * 128, :].bitcast(f32r)
                    )
                    xt.append(t)

                # --- qT / kT projections
                qT, kT = [], []
                for (w_dram, bias, dst) in ((wq_d, bq, qT), (wk_d, bk, kT)):
                    for mt in range(8):
                        wt = pw.tile([128, D], f32r, tag="w")
                        nc.sync.dma_start(out=wt[:], in_=w_dram[b, mt].bitcast(f32r))
                        acc = ps.tile([128, 512], f32, tag="ps")
                        for d in range(8):
                            nc.tensor.matmul(
                                acc[:], wt[:, d * 128 : (d + 1) * 128], xt[d][:],
                                start=(d == 0), stop=(d == 7),
                            )
                        o = (pqT if dst is qT else pkT).tile(
                            [128, S], f32r, tag=("qT" if dst is qT else "kT")
                        )
                        nc.scalar.activation(
                            o[:], acc[:], AF.Identity, bias=bias[:, mt : mt + 1]
                        )
                        dst.append(o)

                # --- v projection (layout [t, 16*(64+1)])
                vt = []
                for t4 in range(4):
                    t = pv.tile([128, 1040], f32r, tag="v")
                    oc = t[:, :].rearrange("p (h c) -> p h c", c=65)[:, :, 64:65]
                    nc.vector.tensor_copy(oc, onesv[:, :].rearrange("p (h c) -> p h c", c=1))
                    vt.append(t)
                for mc in range(2):
                    wv = pwv.tile([128, 4096], f32r, tag="wv")
                    nc.sync.dma_start(out=wv[:], in_=wv_d[b, mc].bitcast(f32r))
                    for t4 in range(4):
                        acc = ps.tile([128, 512], f32, tag="ps")
                        for d in range(8):
                            nc.tensor.matmul(
                                acc[:],
                                xt[d][:, t4 * 128 : (t4 + 1) * 128],
                                wv[:, d * 512 : (d + 1) * 512],
                                start=(d == 0), stop=False,
                            )
                        nc.tensor.matmul(
                            acc[:], ones1[:, 0:128], bv[:, mc * 512 : (mc + 1) * 512],
                            start=False, stop=True,
                        )
                        dst = vt[t4][:, mc * 520 : (mc + 1) * 520].rearrange(
                            "p (h c) -> p h c", c=65
                        )[:, :, 0:64]
                        nc.scalar.copy(dst, acc[:].rearrange("p (h k) -> p h k", k=64))

                # --- attention heads
                mh = [pmh.tile([128, S], f32r, tag="mh", name=f"mh{i}") for i in range(8)]
                for h in range(16):
                    q2, qo = h // 2, (h % 2) * 64
                    qsl = qT[q2][qo : qo + 64, :]
                    ksl = kT[q2][qo : qo + 64, :]
                    ho_ps = ps.tile([65, 512], f32, tag="ps")
                    D_ps = ps.tile([1, 512], f32, tag="ps")
                    for t4 in range(4):
                        sc = ps.tile([128, 512], f32, tag="ps")
                        nc.tensor.matmul(
                            sc[:], ksl[:, t4 * 128 : (t4 + 1) * 128], qsl,
                            start=True, stop=True,
                        )
                        et = pet.tile([128, 512], f32r, tag="et")
                        nc.scalar.activation(et[:], sc[:], AF.Exp, scale=0.125)
                        pr = ppr.tile([128, 512], f32r, tag="pr")
                        nc.vector.tensor_tensor(pr[:], et[:].bitcast(f32), sc[:], ALU.mult)
                        nc.tensor.matmul(
                            D_ps[:], ones128[:], pr[:], start=(t4 == 0), stop=(t4 == 3)
                        )
                        nc.tensor.matmul(
                            ho_ps[:], vt[t4][:, h * 65 : h * 65 + 65], et[:],
                            start=(t4 == 0), stop=(t4 == 3),
                        )
                    hos = phos.tile([65, 512], f32, tag="hos")
                    nc.scalar.copy(hos[:], ho_ps[:])
                    rec = psm.tile([1, 512], f32r, tag="rec")
                    nc.vector.reciprocal(rec[:], hos[64:65, :])
                    R_ps = ps.tile([64, 512], f32, tag="ps")
                    nc.tensor.matmul(R_ps[:], ones1[:, 0:64], rec[:], start=True, stop=True)
                    nc.vector.tensor_tensor(
                        mh[q2][qo : qo + 64, :], hos[0:64, :], R_ps[:], ALU.mult
                    )
                    # ship S and D rows for host-side entropy
                    nc.sync.dma_start(out=ent_d[b, h, 0:1, :], in_=hos[64:65, :])
                    dsb = psm.tile([1, 512], f32, tag="dsb")
                    nc.scalar.copy(dsb[:], D_ps[:])
                    nc.sync.dma_start(out=ent_d[b, h, 1:2, :], in_=dsb[:])

                # --- output projection + residual layernorm, two s-tiles at a time
                for half in range(2):
                    ys = []
                    for si in range(2):
                        st = half * 2 + si
                        for n2 in range(2):
                            yp = ps.tile([128, 512], f32, tag="ps")
                            for mm in range(8):
                                nc.tensor.matmul(
                                    yp[:],
                                    mh[mm][:, st * 128 : (st + 1) * 128],
                                    ow[n2][:, mm * 512 : (mm + 1) * 512],
                                    start=(mm == 0), stop=False,
                                )
                            nc.tensor.matmul(
                                yp[:], ones1[:, 0:128], cob[:, n2 * 512 : (n2 + 1) * 512],
                                start=False, stop=True,
                            )
                            ys.append(yp)
                    for si in range(2):
                        st = half * 2 + si
                        x_t = pxn.tile([128, D], f32, tag="xn")
                        nc.sync.dma_start(out=x_t[:], in_=xn_d[b, st * 128 : (st + 1) * 128, :])
                        hh = pln.tile([128, D], f32, tag="ln")
                        s0 = psm.tile([128, 1], f32, tag="s0")
                        s1 = psm.tile([128, 1], f32, tag="s1")
                        nc.vector.scalar_tensor_tensor(
                            out=hh[:, 0:512], in0=x_t[:, 0:512], scalar=0.0,
                            in1=ys[2 * si][:], op0=ALU.add, op1=ALU.add, accum_out=s0[:],
                        )
                        nc.vector.scalar_tensor_tensor(
                            out=hh[:, 512:1024], in0=x_t[:, 512:1024], scalar=0.0,
                            in1=ys[2 * si + 1][:], op0=ALU.add, op1=ALU.add, accum_out=s1[:],
                        )
                        mu_neg = psm.tile([128, 1], f32, tag="mu")
                        nc.vector.tensor_scalar(
                            out=mu_neg[:], in0=s0[:], scalar1=s1[:], scalar2=-1.0 / D,
                            op0=ALU.add, op1=ALU.mult,
                        )
                        hc = pln.tile([128, D], f32, tag="ln")
                        nc.vector.tensor_scalar_add(hc[:], hh[:], mu_neg[:])
                        sq = pln.tile([128, D], f32, tag="ln")
                        ssq = psm.tile([128, 1], f32, tag="ssq")
                        nc.scalar.activation(sq[:], hc[:], AF.Square, accum_out=ssq[:])
                        sd = psm.tile([128, 1], f32, tag="sd")
                        nc.scalar.activation(sd[:], ssq[:], AF.Sqrt, scale=1.0 / D, bias=eps_t[:])
                        rstd = psm.tile([128, 1], f32, tag="rstd")
                        nc.vector.reciprocal(rstd[:], sd[:])
                        z = pln.tile([128, D], f32, tag="ln")
                        nc.vector.scalar_tensor_tensor(
                            out=z[:], in0=hc[:], scalar=rstd[:], in1=G[:],
                            op0=ALU.mult, op1=ALU.mult,
                        )
                        oo = pln.tile([128, D], f32, tag="ln")
                        nc.vector.tensor_tensor(oo[:], z[:], Bb[:], ALU.add)
                        nc.sync.dma_start(
                            out=out_d[b, st * 128 : (st + 1) * 128, :], in_=oo[:]
                        )

    lp.__exit__(None, None, None)
    nc.finalize()
    return nc


def _get_program(nsamp):
    if nsamp not in _PROGRAM_CACHE:
        _PROGRAM_CACHE[nsamp] = _build_program(nsamp)
    return _PROGRAM_CACHE[nsamp]


# --------------------------------------------------------------- host packing
def _pack_core_inputs(x, top_idx, Wq, bq, Wk, bk, Wv, bv, samples):
    """Build the per-core in_map for the given list of sample indices."""
    ns = len(samples)
    xT = np.empty((ns, D, S), np.float32)
    xn = np.empty((ns, S, D), np.float32)
    wq_p = np.empty((ns, 8, 128, D), np.float32)
    wk_p = np.empty((ns, 8, 128, D), np.float32)
    wv_p = np.empty((ns, 2, 128, 4096), np.float32)
    bq_p = np.empty((ns, 128, 8), np.float32)
    bk_p = np.empty((ns, 128, 8), np.float32)
    bv_p = np.empty((ns, 1, D), np.float32)
    for i, b in enumerate(samples):
        idx = top_idx[b]
        xn[i] = x[b]
        xT[i] = x[b].T
        # gathered, concatenated projection weights [d, 64h+k]
        for (W, bias, wp, bp) in (
            (Wq, bq, wq_p, bq_p), (Wk, bk, wk_p, bk_p),
        ):
            Wc = W[idx].transpose(1, 0, 2).reshape(D, D)
            # pack per m-tile: [mt][p, dd*128+m2] = Wc[dd*128+p, mt*128+m2]
            wp[i] = Wc.reshape(8, 128, 8, 128).transpose(2, 1, 0, 3).reshape(8, 128, D)
            bc = bias[idx].reshape(D)
            bp[i] = bc.reshape(8, 128).T
        Wc = Wv[idx].transpose(1, 0, 2).reshape(D, D)
        # pack per n-half: [mc][p, dd*512+m2] = Wc[dd*128+p, mc*512+m2]
        wv_p[i] = Wc.reshape(8, 128, 2, 512).transpose(2, 1, 0, 3).reshape(2, 128, 4096)
        bv_p[i] = bv[idx].reshape(1, D)
    return {
        "xT": xT, "xn": xn, "wq": wq_p, "wk": wk_p, "wv": wv_p,
        "bq": bq_p, "bk": bk_p, "bv": bv_p,
    }


def kernel(x, u, Wq, bq, Wk, bk, Wv, bv, r1_w, r1_b, ln_g, ln_b,
           r2_w, r2_b, out_w, out_b, norm_g, norm_b):
    from concourse.bass_utils import run_bass_kernel_spmd

    x = np.asarray(x, np.float32)
    top_idx = _router_topk(
        x, np.asarray(u, np.float32), np.asarray(r1_w), np.asarray(r1_b),
        np.asarray(ln_g), np.asarray(ln_b), np.asarray(r2_w), np.asarray(r2_b),
    )

    Wq = np.asarray(Wq, np.float32); bq_h = np.asarray(bq, np.float32)
    Wk = np.asarray(Wk, np.float32); bk_h = np.asarray(bk, np.float32)
    Wv = np.asarray(Wv, np.float32); bv_h = np.asarray(bv, np.float32)
    out_w = np.asarray(out_w, np.float32)

    # shared tensors
    ow_p = out_w.reshape(8, 128, 2, 512).transpose(2, 1, 0, 3).reshape(2, 128, 4096)
    ow_p = np.ascontiguousarray(ow_p)
    c4 = np.zeros((4, D), np.float32)
    c4[0] = np.asarray(out_b, np.float32)
    c4[1] = np.asarray(norm_g, np.float32)
    c4[2] = np.asarray(norm_b, np.float32)

    nc = _get_program(NSAMP)
    in_maps = []
    for c in range(NCORES):
        samples = list(range(c * NSAMP, (c + 1) * NSAMP))
        m = _pack_core_inputs(x, top_idx, Wq, bq_h, Wk, bk_h, Wv, bv_h, samples)
        m["ow"] = ow_p
        m["c4"] = c4
        in_maps.append(m)

    res = run_bass_kernel_spmd(nc, in_maps, core_ids=list(range(NCORES)))

    out = np.empty((B, S, D), np.float32)
    ent_rows = np.empty((B, H, S), np.float64)
    for c in range(NCORES):
        r = res.results[c]
        out[c * NSAMP : (c + 1) * NSAMP] = r["out"]
        ent = r["ent"].astype(np.float64)  # [nsamp, H, 2, S]
        Ssum = ent[:, :, 0, :]
        Draw = ent[:, :, 1, :]
        ent_rows[c * NSAMP : (c + 1) * NSAMP] = np.log(Ssum) - Draw / (8.0 * Ssum)
    avg_entropy = np.float32(ent_rows.mean())
    return out, avg_entropy


# revision 10
# speedup vs baseline: 92.7439x; 92.7439x over previous
"""Bass/Trainium2 kernel for nn_DifferentialAttentionLayer (moe_routing).

Strategy: data-parallel over batch across the 8 NeuronCores (4 samples each).
The tiny router (mean-pool -> 2-layer MLP -> gumbel softmax -> top-16 of a
32-head pool) runs on host CPU with jax to reproduce the reference top_k
bit-exactly; the selected per-sample head weights are gathered/packed on host
and the heavy compute (QKV projections, attention, softmax, entropy, output
projection, residual layernorm) runs on-device in a single SPMD Bass program.

Device-side layout notes (per sample):
  - qT/kT [1024, 512]  (head-major rows: m = 64*h + k), computed as
    Wq[d,m]^T @ xT[d,s] with fp32r matmuls.
  - v [512, 1040]: 16 groups of 65 cols (64 v-cols + a ones col) so the
    "ho" matmul v_aug^T @ expT also produces the softmax row-sum S.
  - attention is computed transposed: scoresT[t,s] = k_h^T q_h, softmax
    without max-subtraction (scores are O(+-3), fp32-exact), entropy via
    ent = ln S - (sum_t e^l * l_raw)/(8 S) shipped to host as S and D rows.
  - mhT [1024, 512] is assembled directly in transposed form so the output
    projection needs no transposes anywhere.
"""

import numpy as np

B, S, D, H, POOL, DK = 32, 512, 1024, 16, 32, 64
NCORES = 8
NSAMP = B // NCORES
TEMP = 0.5

_PROGRAM_CACHE = {}


# ----------------------------------------------------------------- host router
def _router_topk(x, u, r1_w, r1_b, ln_g, ln_b, r2_w, r2_b):
    """Reproduce the reference routing exactly (jax on CPU)."""
    import jax
    import jax.numpy as jnp

    cpu = jax.devices("cpu")[0]
    with jax.default_device(cpu):
        xj = jnp.asarray(x)
        pooled = jnp.mean(xj, axis=1)
        h = pooled @ jnp.asarray(r1_w) + jnp.asarray(r1_b)
        mu = jnp.mean(h, axis=-1, keepdims=True)
        var = jnp.mean((h - mu) ** 2, axis=-1, keepdims=True)
        h = (h - mu) / jnp.sqrt(var + 1e-5) * jnp.asarray(ln_g) + jnp.asarray(ln_b)
        h = jax.nn.relu(h)
        rs = h @ jnp.asarray(r2_w) + jnp.asarray(r2_b)
        gumbel = -jnp.log(-jnp.log(jnp.asarray(u)))
        rw = jax.nn.softmax((rs + gumbel) / TEMP, axis=-1)
        _, idx = jax.lax.top_k(rw, H)
        return np.asarray(idx)


# ------------------------------------------------------------- device program
def _build_program(nsamp):
    import concourse.bacc as bacc
    import concourse.mybir as mybir
    from concourse import tile

    f32 = mybir.dt.float32
    f32r = mybir.dt.float32r
    AF = mybir.ActivationFunctionType
    ALU = mybir.AluOpType

    nc = bacc.Bacc("TRN2", target_bir_lowering=False)

    xT_d = nc.declare_dram_parameter("xT", [nsamp, D, S], f32, isOutput=False)
    id_d = nc.declare_dram_parameter("ident", [128, 128], f32, isOutput=False)
    wq_d = nc.declare_dram_parameter("wq", [nsamp, 8, 128, D], f32, isOutput=False)
    wk_d = nc.declare_dram_parameter("wk", [nsamp, 8, 128, D], f32, isOutput=False)
    wv_d = nc.declare_dram_parameter("wv", [nsamp, 2, 128, 4096], f32, isOutput=False)
    bq_d = nc.declare_dram_parameter("bq", [nsamp, 128, 8], f32, isOutput=False)
    bk_d = nc.declare_dram_parameter("bk", [nsamp, 128, 8], f32, isOutput=False)
    bv_d = nc.declare_dram_parameter("bv", [nsamp, 1, D], f32, isOutput=False)
    ow_d = nc.declare_dram_parameter("ow", [2, 128, 4096], f32, isOutput=False)
    # const4 rows: 0 = out_b, 1 = norm_g, 2 = norm_b
    c4_d = nc.declare_dram_parameter("c4", [4, D], f32, isOutput=False)

    out_d = nc.declare_dram_parameter("out", [nsamp, S, D], f32, isOutput=True)
    ent_d = nc.declare_dram_parameter("ent", [nsamp, H, 2, S], f32, isOutput=True)

    from contextlib import ExitStack

    lp = nc.allow_low_precision("fp32r matmul operands throughout")
    lp.__enter__()
    with tile.TileContext(nc) as tc:
        with ExitStack() as stack:
            pw = stack.enter_context(tc.tile_pool(name="pw", bufs=2))
            pwv = stack.enter_context(tc.tile_pool(name="pwv", bufs=1))
            pow_ = stack.enter_context(tc.tile_pool(name="pow", bufs=2))
            pxT = stack.enter_context(tc.tile_pool(name="pxT", bufs=9))
            pxn = stack.enter_context(tc.tile_pool(name="pxn", bufs=1))
            pqT = stack.enter_context(tc.tile_pool(name="pqT", bufs=8))
            pkT = stack.enter_context(tc.tile_pool(name="pkT", bufs=8))
            pv = stack.enter_context(tc.tile_pool(name="pv", bufs=4))
            pmh = stack.enter_context(tc.tile_pool(name="pmh", bufs=8))
            pet = stack.enter_context(tc.tile_pool(name="pet", bufs=3))
            ppr = stack.enter_context(tc.tile_pool(name="ppr", bufs=2))
            phos = stack.enter_context(tc.tile_pool(name="phos", bufs=2))
            pln = stack.enter_context(tc.tile_pool(name="pln", bufs=3))
            psm = stack.enter_context(tc.tile_pool(name="psm", bufs=2))
            pcn = stack.enter_context(tc.tile_pool(name="pcn", bufs=1))
            ps = stack.enter_context(tc.tile_pool(name="ps", bufs=8, space="PSUM"))
            # ---------------- constants
            ones1_f = pcn.tile([1, 128], f32, tag="ones1f")
            nc.vector.memset(ones1_f[:], 1.0)
            ones1 = pcn.tile([1, 128], f32r, tag="ones1")
            nc.vector.tensor_copy(ones1[:], ones1_f[:])
            ones128_f = pcn.tile([128, 1], f32, tag="o128f")
            nc.vector.memset(ones128_f[:], 1.0)
            ones128 = pcn.tile([128, 1], f32r, tag="o128")
            nc.vector.tensor_copy(ones128[:], ones128_f[:])
            onesv = pcn.tile([128, 16], f32, tag="onesv")
            nc.vector.memset(onesv[:], 1.0)
            eps_t = pcn.tile([128, 1], f32, tag="eps")
            nc.vector.memset(eps_t[:], 1e-5)
            ident = pcn.tile([128, 128], f32, tag="ident")
            nc.sync.dma_start(out=ident[:], in_=id_d[:])

            cob = pcn.tile([1, D], f32r, tag="cob")
            nc.sync.dma_start(out=cob[:], in_=c4_d[0:1, :].bitcast(f32r))
            cng = pcn.tile([1, D], f32r, tag="cng")
            nc.sync.dma_start(out=cng[:], in_=c4_d[1:2, :].bitcast(f32r))
            cnb = pcn.tile([1, D], f32r, tag="cnb")
            nc.sync.dma_start(out=cnb[:], in_=c4_d[2:3, :].bitcast(f32r))

            # G/B broadcast tiles [128, D] of norm_g / norm_b
            G = pcn.tile([128, D], f32, tag="G")
            Bb = pcn.tile([128, D], f32, tag="B")
            for src, dst in ((cng, G), (cnb, Bb)):
                for c2 in range(2):
                    p = ps.tile([128, 512], f32, tag="ps")
                    nc.tensor.matmul(
                        p[:], ones1[:, 0:128], src[:, c2 * 512 : (c2 + 1) * 512],
                        start=True, stop=True,
                    )
                    nc.scalar.copy(dst[:, c2 * 512 : (c2 + 1) * 512], p[:])

            # out_w resident (packed per n-half)
            ow = []
            for n2 in range(2):
                t = pow_.tile([128, 4096], f32r, tag="ow")
                nc.sync.dma_start(out=t[:], in_=ow_d[n2].bitcast(f32r))
                ow.append(t)

            # ---------------- per-sample pipeline
            for b in range(nsamp):
                bq = psm.tile([128, 8], f32, tag="bq")
                nc.sync.dma_start(out=bq[:], in_=bq_d[b])
                bk = psm.tile([128, 8], f32, tag="bk")
                nc.sync.dma_start(out=bk[:], in_=bk_d[b])
                bv = psm.tile([1, D], f32r, tag="bv")
                nc.sync.dma_start(out=bv[:], in_=bv_d[b].bitcast(f32r))

                xt = []
                for d in range(8):
                    t = pxT.tile([128, S], f32r, tag="xT")
                    nc.sync.dma_start(
                        out=t[:], in_=xT_d[b, d * 128 : (d + 1) * 128, :].bitcast(f32r)
                    )
                    xt.append(t)

                # --- qT / kT projections
                qT, kT = [], []
                for (w_dram, bias, dst) in ((wq_d, bq, qT), (wk_d, bk, kT)):
                    for mt in range(8):
                        wt = pw.tile([128, D], f32r, tag="w")
                        nc.sync.dma_start(out=wt[:], in_=w_dram[b, mt].bitcast(f32r))
                        acc = ps.tile([128, 512], f32, tag="ps")
                        for d in range(8):
                            nc.tensor.matmul(
                                acc[:], wt[:, d * 128 : (d + 1) * 128], xt[d][:],
                                start=(d == 0), stop=(d == 7),
                            )
                        o = (pqT if dst is qT else pkT).tile(
                            [128, S], f32r, tag=("qT" if dst is qT else "kT")
                        )
                        nc.scalar.activation(
                            o[:], acc[:], AF.Identity, bias=bias[:, mt : mt + 1]
                        )
                        dst.append(o)

                # --- v projection (layout [t, 16*(64+1)])
                vt = []
                for t4 in range(4):
                    t = pv.tile([128, 1040], f32r, tag="v")
                    oc = t[:, :].rearrange("p (h c) -> p h c", c=65)[:, :, 64:65]
                    nc.vector.tensor_copy(oc, onesv[:, :].rearrange("p (h c) -> p h c", c=1))
                    vt.append(t)
                for mc in range(2):
                    wv = pwv.tile([128, 4096], f32r, tag="wv")
                    nc.sync.dma_start(out=wv[:], in_=wv_d[b, mc].bitcast(f32r))
                    for t4 in range(4):
                        acc = ps.tile([128, 512], f32, tag="ps")
                        for d in range(8):
                            nc.tensor.matmul(
                                acc[:],
                                xt[d][:, t4 * 128 : (t4 + 1) * 128],
                                wv[:, d * 512 : (d + 1) * 512],
                                start=(d == 0), stop=False,
                            )
                        nc.tensor.matmul(
                            acc[:], ones1[:, 0:128], bv[:, mc * 512 : (mc + 1) * 512],
                            start=False, stop=True,
                        )
                        dst = vt[t4][:, mc * 520 : (mc + 1) * 520].rearrange(
                            "p (h c) -> p h c", c=65
                        )[:, :, 0:64]
                        nc.scalar.copy(dst, acc[:].rearrange("p (h k) -> p h k", k=64))

                # --- attention heads
                mh = [pmh.tile([128, S], f32r, tag="mh", name=f"mh{i}") for i in range(8)]
                for h in range(16):
                    q2, qo = h // 2, (h % 2) * 64
                    qsl = qT[q2][qo : qo + 64, :]
                    ksl = kT[q2][qo : qo + 64, :]
                    ho_ps = ps.tile([65, 512], f32, tag="ps")
                    D_ps = ps.tile([1, 512], f32, tag="ps")
                    for t4 in range(4):
                        sc = ps.tile([128, 512], f32, tag="ps")
                        nc.tensor.matmul(
                            sc[:], ksl[:, t4 * 128 : (t4 + 1) * 128], qsl,
                            start=True, stop=True,
                        )
                        et = pet.tile([128, 512], f32r, tag="et")
                        nc.scalar.activation(et[:], sc[:], AF.Exp, scale=0.125)
                        pr = ppr.tile([128, 512], f32r, tag="pr")
                        nc.vector.tensor_tensor(pr[:], et[:].bitcast(f32), sc[:], ALU.mult)
                        nc.tensor.matmul(
                            D_ps[:], ones128[:], pr[:], start=(t4 == 0), stop=(t4 == 3)
                        )
                        nc.tensor.matmul(
                            ho_ps[:], vt[t4][:, h * 65 : h * 65 + 65], et[:],
                            start=(t4 == 0), stop=(t4 == 3),
                        )
                    hos = phos.tile([65, 512], f32, tag="hos")
                    nc.scalar.copy(hos[:], ho_ps[:])
                    rec = psm.tile([1, 512], f32r, tag="rec")
                    nc.vector.reciprocal(rec[:], hos[64:65, :])
                    R_ps = ps.tile([64, 512], f32, tag="ps")
                    nc.tensor.matmul(R_ps[:], ones1[:, 0:64], rec[:], start=True, stop=True)
                    nc.vector.tensor_tensor(
                        mh[q2][qo : qo + 64, :], hos[0:64, :], R_ps[:], ALU.mult
                    )
                    # ship S and D rows for host-side entropy
                    nc.sync.dma_start(out=ent_d[b, h, 0:1, :], in_=hos[64:65, :])
                    dsb = psm.tile([1, 512], f32, tag="dsb")
                    nc.scalar.copy(dsb[:], D_ps[:])
                    nc.sync.dma_start(out=ent_d[b, h, 1:2, :], in_=dsb[:])

                # --- output projection + residual layernorm, two s-tiles at a time
                for half in range(2):
                    ys = []
                    for si in range(2):
                        st = half * 2 + si
                        for n2 in range(2):
                            yp = ps.tile([128, 512], f32, tag="ps")
                            for mm in range(8):
                                nc.tensor.matmul(
                                    yp[:],
                                    mh[mm][:, st * 128 : (st + 1) * 128],
                                    ow[n2][:, mm * 512 : (mm + 1) * 512],
                                    start=(mm == 0), stop=False,
                                )
                            nc.tensor.matmul(
                                yp[:], ones1[:, 0:128], cob[:, n2 * 512 : (n2 + 1) * 512],
                                start=False, stop=True,
                            )
                            ys.append(yp)
                    for si in range(2):
                        st = half * 2 + si
                        x_t = pxn.tile([128, D], f32, tag="xn")
                        for dd in range(8):
                            xps = ps.tile([128, 128], f32, tag="ps")
                            nc.tensor.transpose(
                                xps[:], xt[dd][:, st * 128 : (st + 1) * 128].bitcast(f32),
                                ident[:],
                            )
                            nc.scalar.copy(x_t[:, dd * 128 : (dd + 1) * 128], xps[:])
                        hh = pln.tile([128, D], f32, tag="ln")
                        s0 = psm.tile([128, 1], f32, tag="s0")
                        s1 = psm.tile([128, 1], f32, tag="s1")
                        nc.vector.scalar_tensor_tensor(
                            out=hh[:, 0:512], in0=x_t[:, 0:512], scalar=0.0,
                            in1=ys[2 * si][:], op0=ALU.add, op1=ALU.add, accum_out=s0[:],
                        )
                        nc.vector.scalar_tensor_tensor(
                            out=hh[:, 512:1024], in0=x_t[:, 512:1024], scalar=0.0,
                            in1=ys[2 * si + 1][:], op0=ALU.add, op1=ALU.add, accum_out=s1[:],
                        )
                        mu_neg = psm.tile([128, 1], f32, tag="mu")
                        nc.vector.tensor_scalar(
                            out=mu_neg[:], in0=s0[:], scalar1=s1[:], scalar2=-1.0 / D,
                            op0=ALU.add, op1=ALU.mult,
                        )
                        hc = pln.tile([128, D], f32, tag="ln")
                        nc.vector.tensor_scalar_add(hc[:], hh[:], mu_neg[:])
                        sq = pln.tile([128, D], f32, tag="ln")
                        ssq = psm.tile([128, 1], f32, tag="ssq")
                        nc.scalar.activation(sq[:], hc[:], AF.Square, accum_out=ssq[:])
                        sd = psm.tile([128, 1], f32, tag="sd")
                        nc.scalar.activation(sd[:], ssq[:], AF.Sqrt, scale=1.0 / D, bias=eps_t[:])
                        rstd = psm.tile([128, 1], f32, tag="rstd")
                        nc.vector.reciprocal(rstd[:], sd[:])
                        z = pln.tile([128, D], f32, tag="ln")
                        nc.vector.scalar_tensor_tensor(
                            out=z[:], in0=hc[:], scalar=rstd[:], in1=G[:],
                            op0=ALU.mult, op1=ALU.mult,
                        )
                        oo = pln.tile([128, D], f32, tag="ln")
                        nc.vector.tensor_tensor(oo[:], z[:], Bb[:], ALU.add)
                        nc.sync.dma_start(
                            out=out_d[b, st * 128 : (st + 1) * 128, :], in_=oo[:]
                        )

    lp.__exit__(None, None, None)
    nc.finalize()
    return nc


def _get_program(nsamp):
    if nsamp not in _PROGRAM_CACHE:
        _PROGRAM_CACHE[nsamp] = _build_program(nsamp)
    return _PROGRAM_CACHE[nsamp]


# --------------------------------------------------------------- host packing
def _pack_core_inputs(x, top_idx, Wq, bq, Wk, bk, Wv, bv, samples):
    """Build the per-core in_map for the given list of sample indices."""
    ns = len(samples)
    xT = np.empty((ns, D, S), np.float32)
    wq_p = np.empty((ns, 8, 128, D), np.float32)
    wk_p = np.empty((ns, 8, 128, D), np.float32)
    wv_p = np.empty((ns, 2, 128, 4096), np.float32)
    bq_p = np.empty((ns, 128, 8), np.float32)
    bk_p = np.empty((ns, 128, 8), np.float32)
    bv_p = np.empty((ns, 1, D), np.float32)
    for i, b in enumerate(samples):
        idx = top_idx[b]
        xT[i] = x[b].T
        # gathered, concatenated projection weights [d, 64h+k]
        for (W, bias, wp, bp) in (
            (Wq, bq, wq_p, bq_p), (Wk, bk, wk_p, bk_p),
        ):
            Wc = W[idx].transpose(1, 0, 2).reshape(D, D)
            # pack per m-tile: [mt][p, dd*128+m2] = Wc[dd*128+p, mt*128+m2]
            wp[i] = Wc.reshape(8, 128, 8, 128).transpose(2, 1, 0, 3).reshape(8, 128, D)
            bc = bias[idx].reshape(D)
            bp[i] = bc.reshape(8, 128).T
        Wc = Wv[idx].transpose(1, 0, 2).reshape(D, D)
        # pack per n-half: [mc][p, dd*512+m2] = Wc[dd*128+p, mc*512+m2]
        wv_p[i] = Wc.reshape(8, 128, 2, 512).transpose(2, 1, 0, 3).reshape(2, 128, 4096)
        bv_p[i] = bv[idx].reshape(1, D)
    return {
        "xT": xT, "wq": wq_p, "wk": wk_p, "wv": wv_p,
        "bq": bq_p, "bk": bk_p, "bv": bv_p,
    }


_RUNNER_CACHE = {}


def _get_runner(nsamp):
    """Cached jitted SPMD executor mirroring bass2jax.run_bass_via_pjrt's
    multi-core path (stable jit => no per-call XLA recompile; zero output
    buffers are created device-side instead of being uploaded)."""
    if nsamp in _RUNNER_CACHE:
        return _RUNNER_CACHE[nsamp]
    import jax
    import jax.numpy as jnp
    import concourse.mybir as mybir
    from concourse import bass2jax
    from jax.sharding import Mesh, NamedSharding, PartitionSpec
    from jax.experimental.shard_map import shard_map

    nc = _get_program(nsamp)
    bass2jax.install_neuronx_cc_hook()
    partition_name = nc.partition_id_tensor.name if nc.partition_id_tensor else None
    in_names, out_names, out_avals = [], [], []
    for alloc in nc.m.functions[0].allocations:
        if not isinstance(alloc, mybir.MemoryLocationSet):
            continue
        name = alloc.memorylocations[0].name
        if alloc.kind == "ExternalInput":
            if name != partition_name:
                in_names.append(name)
        elif alloc.kind == "ExternalOutput":
            out_avals.append(
                jax.core.ShapedArray(tuple(alloc.tensor_shape), mybir.dt.np(alloc.dtype))
            )
            out_names.append(name)
    n_params = len(in_names)
    n_outs = len(out_names)
    full_in_names = list(in_names) + list(out_names)
    if partition_name is not None:
        full_in_names.append(partition_name)

    def _body(*args):
        operands = list(args)
        if partition_name is not None:
            operands.append(bass2jax.partition_id_tensor())
        outs = bass2jax._bass_exec_p.bind(
            *operands,
            out_avals=tuple(out_avals),
            in_names=tuple(full_in_names),
            out_names=tuple(out_names),
            lowering_input_output_aliases=(),
            sim_require_finite=True,
            sim_require_nnan=True,
            nc=nc,
        )
        return tuple(outs)

    devices = jax.devices()[:NCORES]
    mesh = Mesh(np.asarray(devices), ("core",))
    sharded = jax.jit(
        shard_map(
            _body, mesh=mesh,
            in_specs=(PartitionSpec("core"),) * (n_params + n_outs),
            out_specs=(PartitionSpec("core"),) * n_outs,
            check_rep=False,
        ),
        donate_argnums=tuple(range(n_params, n_params + n_outs)),
        keep_unused=True,
    )
    shard = NamedSharding(mesh, PartitionSpec("core"))
    runner = {
        "sharded": sharded, "in_names": in_names, "out_names": out_names,
        "out_avals": out_avals, "shard": shard, "jnp": jnp, "jax": jax,
    }
    _RUNNER_CACHE[nsamp] = runner
    return runner


def _upload_inputs(runner, in_maps):
    """Concat per-core inputs along axis 0 and place them on the mesh."""
    import jax
    globs = []
    for name in runner["in_names"]:
        g = np.concatenate([np.asarray(m[name]) for m in in_maps], axis=0)
        globs.append(jax.device_put(g, runner["shard"]))
    return globs


def _device_zeros(runner):
    jnp = runner["jnp"]
    zs = []
    for av in runner["out_avals"]:
        zs.append(jnp.zeros((NCORES * av.shape[0],) + av.shape[1:], av.dtype,
                            device=runner["shard"]))
    return zs


def _run_spmd(runner, in_args):
    out_arrs = runner["sharded"](*in_args, *_device_zeros(runner))
    results = []
    for c in range(NCORES):
        r = {}
        for i, name in enumerate(runner["out_names"]):
            av = runner["out_avals"][i]
            r[name] = np.asarray(out_arrs[i]).reshape((NCORES,) + av.shape)[c]
        results.append(r)
    return results


def kernel(x, u, Wq, bq, Wk, bk, Wv, bv, r1_w, r1_b, ln_g, ln_b,
           r2_w, r2_b, out_w, out_b, norm_g, norm_b):
    x = np.asarray(x, np.float32)
    top_idx = _router_topk(
        x, np.asarray(u, np.float32), np.asarray(r1_w), np.asarray(r1_b),
        np.asarray(ln_g), np.asarray(ln_b), np.asarray(r2_w), np.asarray(r2_b),
    )

    Wq = np.asarray(Wq, np.float32); bq_h = np.asarray(bq, np.float32)
    Wk = np.asarray(Wk, np.float32); bk_h = np.asarray(bk, np.float32)
    Wv = np.asarray(Wv, np.float32); bv_h = np.asarray(bv, np.float32)
    out_w = np.asarray(out_w, np.float32)

    # shared tensors
    ow_p = out_w.reshape(8, 128, 2, 512).transpose(2, 1, 0, 3).reshape(2, 128, 4096)
    ow_p = np.ascontiguousarray(ow_p)
    c4 = np.zeros((4, D), np.float32)
    c4[0] = np.asarray(out_b, np.float32)
    c4[1] = np.asarray(norm_g, np.float32)
    c4[2] = np.asarray(norm_b, np.float32)

    ident = np.eye(128, dtype=np.float32)
    runner = _get_runner(NSAMP)
    in_maps = []
    for c in range(NCORES):
        samples = list(range(c * NSAMP, (c + 1) * NSAMP))
        m = _pack_core_inputs(x, top_idx, Wq, bq_h, Wk, bk_h, Wv, bv_h, samples)
        m["ow"] = ow_p
        m["c4"] = c4
        m["ident"] = ident
        in_maps.append(m)

    results = _run_spmd(runner, _upload_inputs(runner, in_maps))

    out = np.empty((B, S, D), np.float32)
    ent_rows = np.empty((B, H, S), np.float64)
    for c in range(NCORES):
        r = results[c]
        out[c * NSAMP : (c + 1) * NSAMP] = r["out"]
        ent = r["ent"].astype(np.float64)  # [nsamp, H, 2, S]
        Ssum = ent[:, :, 0, :]
        Draw = ent[:, :, 1, :]
        ent_rows[c * NSAMP : (c + 1) * NSAMP] = np.log(Ssum) - Draw / (8.0 * Ssum)
    avg_entropy = np.float32(ent_rows.mean())
    return out, avg_entropy
